# revision 30
# baseline (speedup 1.0000x reference)
"""DynamicMoE (B=4, S=2048, D=1024, E=8, H=4096, top-2) on 8 trn2 cores.

Key observation: the reference loops experts in index order and OVERWRITES
(out = where(w_i>0, y_i, out)), so each token's final output comes from the
single highest-indexed expert of its top-2. Each token therefore needs exactly
one expert MLP, with its input pre-scaled by that expert's softmax score.

Strategy (expert-parallel with host-side routing):
- Host: route in f64, scale+permute tokens by expert, pack (expert, token
  block) parts into 8 cores x NSLOT weight slots via a small DP packer.
- Device (one SPMD program): per slot, a 2-layer MLP in float32r (full PE
  rate, ~1.5e-4 rel err) with tokens on the matmul moving dim, weights
  streamed from HBM in pre-transposed layouts so all DMAs are contiguous.
"""

import numpy as np

_B, _S, _D, _E, _H = 4, 2048, 1024, 8, 4096
_N = _B * _S
_KD = _D // 128   # 8 d-chunks (layer-1 contraction / layer-2 output)
_HI = _H // 128   # 32 h-chunks


def _chunks(L):
    """Split L (>=256) into matmul chunks of 256..496 tokens: >=256 so
    float32r runs at full PE rate, <=496 since exactly-full PSUM banks
    (N=512) measured ~8% slower per column than N<=480."""
    sizes = []
    rem = L
    while rem > 752:
        sizes.append(496)
        rem -= 496
    if rem > 496:
        sizes += [256, rem - 256]
    else:
        sizes.append(rem)
    pos, t0 = [], 0
    for c in sizes:
        pos.append((t0, c))
        t0 += c
    return pos


def _route(x, gate_w, gate_b):
    """Per-token (expert, scale): the higher-indexed of the top-2 experts and
    its softmax score. f64 to track the f32 reference's ordering closely."""
    xf = x.reshape(_N, _D).astype(np.float64)
    logits = xf @ gate_w.astype(np.float64).T + gate_b.astype(np.float64)
    # jax.lax.top_k tie-break: smaller index first -> stable descending sort
    top2 = np.argsort(-logits, axis=1, kind="stable")[:, :2]
    e_sel = top2.max(axis=1)
    m = logits.max(axis=1, keepdims=True)
    p = np.exp(logits - m)
    p /= p.sum(axis=1, keepdims=True)
    scale = p[np.arange(_N), e_sel]
    return e_sel.astype(np.int64), scale.astype(np.float32)


def _pack(counts, slot_sizes_list):
    """Pick a slot structure and assign experts to (core, slot) parts.

    Each candidate is a list of per-core slot sizes [s0, s1, ...]; every core
    runs the same structure, 8 slots of each size exist in total. Feasibility
    via DP over how many slots of each size every expert consumes.
    Returns (slot_sizes, parts) with parts = list of (expert, count, core,
    slot_idx); count <= slot size, zero-padded on device.
    """
    experts = [e for e in range(_E) if counts[e] > 0]

    from itertools import product

    best = None
    for sizes in slot_sizes_list:
        nslot = len(sizes)
        # options per expert: tuples (k_0..k_{nslot-1}) with sum(k_i*s_i) >= n
        def expert_opts(n):
            opts = [
                t for t in product(range(9), repeat=nslot)
                if sum(k * s for k, s in zip(t, sizes)) >= n
            ]
            return sorted(opts, key=lambda t: (sum(t), t))[:64]

        states = {tuple([0] * nslot): []}
        ok = True
        for e in experts:
            nxt = {}
            for opt in expert_opts(int(counts[e])):
                for st, hist in states.items():
                    ns = tuple(a + b for a, b in zip(st, opt))
                    if all(v <= 8 for v in ns) and ns not in nxt:
                        nxt[ns] = hist + [(e, opt)]
            if not nxt:
                ok = False
                break
            states = nxt
        if not ok:
            continue
        alloc = min(states.values(), key=lambda h: 0)  # any feasible
        cap = sum(sizes)
        pe_ns = cap * 218  # ~213ns/token PE + margin
        dma_ns = (nslot * 32e6 + cap * 8192) / 358e9 * 1e9
        score = max(pe_ns, dma_ns)
        if best is None or score < best[0]:
            best = (score, sizes, alloc)

    assert best is not None, f"no feasible slot structure for counts={counts}"
    _, sizes, alloc = best

    # materialize parts: per slot-kind, hand out slot indices core 0..7
    next_core = [0] * len(sizes)
    parts = []
    for e, opt in alloc:
        rem = int(counts[e])
        # fill largest slots first
        order = sorted(range(len(sizes)), key=lambda i: -sizes[i])
        for i in order:
            for _ in range(opt[i]):
                take = max(0, min(rem, sizes[i]))
                core = next_core[i]
                next_core[i] += 1
                parts.append((e, take, core, i))
                rem -= take
        assert rem <= 0
    return list(sizes), parts


_PROG_CACHE = {}


def _build_program(slot_sizes):
    """One SPMD Bass program for all 8 cores, parameterized by slot sizes."""
    import concourse.tile as tile
    from concourse import bacc, mybir

    key = tuple(slot_sizes)
    if key in _PROG_CACHE:
        return _PROG_CACHE[key]

    F32 = mybir.dt.float32
    F32R = mybir.dt.float32r
    CAP = sum(slot_sizes)
    nslot = len(slot_sizes)

    nc = bacc.Bacc("TRN2", target_bir_lowering=False, debug=False, num_devices=8)
    xt = nc.dram_tensor("xt", [128, _KD, CAP], F32, kind="ExternalInput").ap()
    w1d = [
        nc.dram_tensor(f"w1_{s}", [_HI, 128, _KD, 128], F32, kind="ExternalInput").ap()
        for s in range(nslot)
    ]
    w2d = [
        nc.dram_tensor(f"w2_{s}", [_KD, 128, _HI, 128], F32, kind="ExternalInput").ap()
        for s in range(nslot)
    ]
    b1d = [
        nc.dram_tensor(f"b1_{s}", [128, _HI], F32, kind="ExternalInput").ap()
        for s in range(nslot)
    ]
    b2d = [
        nc.dram_tensor(f"b2_{s}", [128, _KD], F32, kind="ExternalInput").ap()
        for s in range(nslot)
    ]
    outT = nc.dram_tensor("outT", [_KD, 128, CAP], F32, kind="ExternalOutput").ap()

    Relu = mybir.ActivationFunctionType.Relu
    Ident = mybir.ActivationFunctionType.Identity

    max_cks = max(len(_chunks(Ls)) for Ls in slot_sizes)
    offs = [0]
    for Ls in slot_sizes:
        offs.append(offs[-1] + Ls)
    with tile.TileContext(nc) as tc:
        with tc.tile_pool(name="xp", bufs=max(2, max_cks)) as xp, \
             tc.tile_pool(name="w1p", bufs=5) as w1p, \
             tc.tile_pool(name="w1f", bufs=2) as w1f, \
             tc.tile_pool(name="w2p", bufs=11) as w2p, \
             tc.tile_pool(name="h1p", bufs=1) as h1p, \
             tc.tile_pool(name="cp", bufs=2) as cp, \
             tc.tile_pool(name="op", bufs=4) as op, \
             tc.tile_pool(name="ps1", bufs=4, space="PSUM") as ps1, \
             tc.tile_pool(name="ps2", bufs=4, space="PSUM") as ps2:

            slot_pre = {}  # s -> (x tiles, w1-hi0 tile, b1 tile, b2 tile)

            def emit_slot_prefetch(s):
                """Queue slot s's x chunks, first w1 block, and biases on the
                sync ring. Order: x ck0, w1 hi0, biases, remaining x chunks —
                so the first matmul of the slot waits on as little as
                possible. For slot 0 (kernel startup, DMA-starved) the later
                x chunks are deferred and interleaved as k-halves between the
                first w1 blocks inside the L1 loop."""
                off = offs[s]
                cks = _chunks(slot_sizes[s])
                xc0 = xp.tile([128, _KD, cks[0][1]], F32R, tag="x")
                nc.sync.dma_start(
                    xc0[:], xt[:, :, off:off + cks[0][1]].bitcast(F32R)
                )
                w1_0 = w1f.tile([128, _KD, 128], F32R, tag="w1first")
                nc.sync.dma_start(w1_0[:], w1d[s][0].bitcast(F32R))
                b1_sb = cp.tile([128, _HI], F32, tag="b1")
                nc.sync.dma_start(b1_sb[:], b1d[s][:])
                b2_sb = cp.tile([128, _KD], F32, tag="b2")
                nc.sync.dma_start(b2_sb[:], b2d[s][:])
                xs = [xc0]
                pending = {}  # hi -> [(sbuf slice, dram slice)] for slot 0
                half = _KD // 2
                for ci, (t0, tl) in enumerate(cks[1:], start=1):
                    xc = xp.tile([128, _KD, tl], F32R, tag="x")
                    src = xt[:, :, off + t0:off + t0 + tl].bitcast(F32R)
                    if s == 0:
                        pending.setdefault(2 * ci - 1, []).append(
                            (xc[:, 0:half, :], src[:, 0:half, :])
                        )
                        pending.setdefault(2 * ci, []).append(
                            (xc[:, half:_KD, :], src[:, half:_KD, :])
                        )
                    else:
                        nc.sync.dma_start(xc[:], src)
                    xs.append(xc)
                slot_pre[s] = (xs, w1_0, b1_sb, b2_sb, pending)

            emit_slot_prefetch(0)
            for s, Ls in enumerate(slot_sizes):
                off = offs[s]
                cks = _chunks(Ls)
                x_sb, w1_first, b1_sb, b2_sb, pending_x = slot_pre.pop(s)

                h1_sb = h1p.tile([128, _HI, Ls], F32R, tag="h1")
                # (hi=0, chunk>0) runs at the END of layer 1: at slot start
                # only chunk 0's x has landed, so starting with (0,0) alone
                # avoids a PE stall waiting for the later x chunks
                if s == 0 and len(cks) == 2:
                    # startup is DMA-starved: run chunk 1 two h-blocks behind
                    # chunk 0 so its deferred x halves (emitted at hi=1,2)
                    # land before first use
                    l1_iter = [(0, 0), (1, 0), (2, 0)]
                    for hi in range(3, _HI):
                        l1_iter += [(hi, 0), (hi - 2, 1)]
                    l1_iter += [(_HI - 2, 1), (_HI - 1, 1), (0, 1)]
                else:
                    l1_iter = [(0, 0)]
                    l1_iter += [(hi, ci) for hi in range(1, _HI)
                                for ci in range(len(cks))]
                    l1_iter += [(0, ci) for ci in range(1, len(cks))]
                w1_tiles = {0: w1_first}
                for hi, ci in l1_iter:
                    if hi not in w1_tiles:
                        w1_sb = w1p.tile([128, _KD, 128], F32R, tag="w1")
                        nc.sync.dma_start(w1_sb[:], w1d[s][hi].bitcast(F32R))
                        w1_tiles[hi] = w1_sb
                        # slot 0: deferred x-chunk halves ride between the
                        # first w1 blocks so the PE is never starved
                        for dst, src in pending_x.pop(hi, ()):
                            nc.sync.dma_start(dst, src)
                    w1_sb = w1_tiles[hi]
                    t0, tl = cks[ci]
                    ps = ps1.tile([128, tl], F32, tag="ps1")
                    for k in range(_KD):
                        nc.tensor.matmul(
                            ps[:], w1_sb[:, k, :], x_sb[ci][:, k, :],
                            start=(k == 0), stop=(k == _KD - 1),
                        )
                    nc.scalar.activation(
                        h1_sb[:, hi, t0:t0 + tl], ps[:], Relu,
                        bias=b1_sb[:, hi:hi + 1],
                    )

                for di in range(_KD):
                    # w2 streamed in quarter blocks (8 h-chunks each) so the
                    # hi-loop only waits on the quarter it needs
                    w2_sb = []
                    for q in range(4):
                        wq = w2p.tile([128, 8, 128], F32R, tag="w2")
                        nc.sync.dma_start(
                            wq[:], w2d[s][di, :, 8 * q:8 * q + 8, :].bitcast(F32R)
                        )
                        w2_sb.append(wq)
                    if di == _KD - 1 and s + 1 < nslot:
                        # all of this slot's weights are queued; prefetch the
                        # next slot's x/biases/first-w1 behind them
                        emit_slot_prefetch(s + 1)
                    for (t0, tl) in cks:
                        ps = ps2.tile([128, tl], F32, tag="ps2")
                        for hi in range(_HI):
                            nc.tensor.matmul(
                                ps[:], w2_sb[hi // 8][:, hi % 8, :],
                                h1_sb[:, hi, t0:t0 + tl],
                                start=(hi == 0), stop=(hi == _HI - 1),
                            )
                        ob = op.tile([128, tl], F32, tag="ob")
                        nc.scalar.activation(
                            ob[:], ps[:], Ident, bias=b2_sb[:, di:di + 1],
                        )
                        nc.sync.dma_start(
                            outT[di, :, off + t0:off + t0 + tl], ob[:]
                        )

    nc.compile()
    _PROG_CACHE[key] = nc
    return nc


def _run(x, gate_w, gate_b, w1, b1, w2, b2, trace=False, trace_cores=None):
    from concourse import bass_utils

    e_sel, scale = _route(x, gate_w, gate_b)
    counts = np.bincount(e_sel, minlength=_E)

    # candidate structures: all 2-slot (A>=B), plus guaranteed 3-slot fallbacks
    cands = []
    for A in range(256, 705, 16):
        for Bv in range(256, A + 1, 16):
            cands.append([A, Bv])
    cands.append([512, 512, 512])
    cands.append([704, 704, 704])
    slot_sizes, parts = _pack(counts, cands)
    CAP = sum(slot_sizes)

    # token ids per expert in sorted order
    order = np.argsort(e_sel, kind="stable")
    starts = np.zeros(_E + 1, np.int64)
    np.cumsum(counts, out=starts[1:])
    consumed = [0] * _E

    # slot offsets within a core's token axis
    offs = np.zeros(len(slot_sizes) + 1, np.int64)
    np.cumsum(slot_sizes, out=offs[1:])

    xs = x.reshape(_N, _D) * scale[:, None]  # f32, matches reference scaling

    # prearranged weights, one contiguous block per (expert, chunk):
    # W1L[e, hi, p, k, f] = w1[e, hi*128+f, k*128+p]
    W1L = np.ascontiguousarray(
        w1.reshape(_E, _HI, 128, _KD, 128).transpose(0, 1, 4, 3, 2)
    )
    # W2L[e, di, p, hi, f] = w2[e, di*128+f, hi*128+p]
    W2L = np.ascontiguousarray(
        w2.reshape(_E, _KD, 128, _HI, 128).transpose(0, 1, 4, 3, 2)
    )
    B1L = np.ascontiguousarray(b1.reshape(_E, _HI, 128).transpose(0, 2, 1))
    B2L = np.ascontiguousarray(b2.reshape(_E, _KD, 128).transpose(0, 2, 1))

    slot_expert = [[0] * len(slot_sizes) for _ in range(8)]
    tok_of = np.full((8, CAP), -1, np.int64)
    for (e, cnt, core, si) in parts:
        lo = starts[e] + consumed[e]
        consumed[e] += cnt
        toks = order[lo:lo + cnt]
        tok_of[core, offs[si]:offs[si] + cnt] = toks
        slot_expert[core][si] = e

    in_maps = []
    for core in range(8):
        cols = tok_of[core]
        xsel = np.zeros((CAP, _D), np.float32)
        valid = cols >= 0
        xsel[valid] = xs[cols[valid]]
        XL = np.ascontiguousarray(
            xsel.reshape(CAP, _KD, 128).transpose(2, 1, 0)
        )
        m = {"xt": XL}
        for si in range(len(slot_sizes)):
            e = slot_expert[core][si]
            m[f"w1_{si}"] = W1L[e]
            m[f"w2_{si}"] = W2L[e]
            m[f"b1_{si}"] = B1L[e]
            m[f"b2_{si}"] = B2L[e]
        in_maps.append(m)

    nc = _build_program(slot_sizes)
    kw = {}
    if trace:
        kw["trace"] = True
        if trace_cores is not None:
            kw["trace_cores"] = trace_cores
    try:
        res = bass_utils.run_bass_kernel_spmd(
            nc, in_maps, core_ids=list(range(8)), **kw
        )
    except Exception:
        # one retry for transient device faults
        import time as _time
        _time.sleep(2.0)
        res = bass_utils.run_bass_kernel_spmd(
            nc, in_maps, core_ids=list(range(8)), **kw
        )

    out = np.zeros((_N, _D), np.float32)
    for core in range(8):
        cols = tok_of[core]
        valid = cols >= 0
        oc = res.results[core]["outT"]  # [KD, 128, CAP]
        ovals = oc.transpose(2, 0, 1).reshape(CAP, _D)
        out[cols[valid]] = ovals[valid]
    return out.reshape(_B, _S, _D), res


def kernel(x, gate_w, gate_b, w1, b1, w2, b2):
    x = np.ascontiguousarray(np.asarray(x, dtype=np.float32))
    gate_w = np.asarray(gate_w, dtype=np.float32)
    gate_b = np.asarray(gate_b, dtype=np.float32)
    w1 = np.ascontiguousarray(np.asarray(w1, dtype=np.float32))
    b1 = np.asarray(b1, dtype=np.float32)
    w2 = np.ascontiguousarray(np.asarray(w2, dtype=np.float32))
    b2 = np.asarray(b2, dtype=np.float32)
    out, _ = _run(x, gate_w, gate_b, w1, b1, w2, b2)
    return out


# revision 33
# speedup vs baseline: 1.0700x; 1.0700x over previous
"""DynamicMoE (B=4, S=2048, D=1024, E=8, H=4096, top-2) on 8 trn2 cores.

Key observation: the reference loops experts in index order and OVERWRITES
(out = where(w_i>0, y_i, out)), so each token's final output comes from the
single highest-indexed expert of its top-2. Each token therefore needs exactly
one expert MLP, with its input pre-scaled by that expert's softmax score.

Strategy (expert-parallel with host-side routing):
- Host: route in f64, scale+permute tokens by expert, pack (expert, token
  block) parts into 8 cores x NSLOT weight slots via a small DP packer.
- Device (one SPMD program): per slot, a 2-layer MLP in float32r (full PE
  rate, ~1.5e-4 rel err) with tokens on the matmul moving dim, weights
  streamed from HBM in pre-transposed layouts so all DMAs are contiguous.
"""

import numpy as np

_B, _S, _D, _E, _H = 4, 2048, 1024, 8, 4096
_N = _B * _S
_KD = _D // 128   # 8 d-chunks (layer-1 contraction / layer-2 output)
_HI = _H // 128   # 32 h-chunks


def _chunks(L):
    """Split L (>=256) into matmul chunks of 256..496 tokens: >=256 so
    float32r runs at full PE rate, <=496 since exactly-full PSUM banks
    (N=512) measured ~8% slower per column than N<=480."""
    sizes = []
    rem = L
    while rem > 752:
        sizes.append(496)
        rem -= 496
    if rem > 496:
        sizes += [256, rem - 256]
    else:
        sizes.append(rem)
    pos, t0 = [], 0
    for c in sizes:
        pos.append((t0, c))
        t0 += c
    return pos


def _route(x, gate_w, gate_b):
    """Per-token (expert, scale): the higher-indexed of the top-2 experts and
    its softmax score. f64 to track the f32 reference's ordering closely."""
    xf = x.reshape(_N, _D).astype(np.float64)
    logits = xf @ gate_w.astype(np.float64).T + gate_b.astype(np.float64)
    # jax.lax.top_k tie-break: smaller index first -> stable descending sort
    top2 = np.argsort(-logits, axis=1, kind="stable")[:, :2]
    e_sel = top2.max(axis=1)
    m = logits.max(axis=1, keepdims=True)
    p = np.exp(logits - m)
    p /= p.sum(axis=1, keepdims=True)
    scale = p[np.arange(_N), e_sel]
    return e_sel.astype(np.int64), scale.astype(np.float32)


def _pack(counts, slot_sizes_list):
    """Pick a slot structure and assign experts to (core, slot) parts.

    Each candidate is a list of per-core slot sizes [s0, s1, ...]; every core
    runs the same structure, 8 slots of each size exist in total. Feasibility
    via DP over how many slots of each size every expert consumes.
    Returns (slot_sizes, parts) with parts = list of (expert, count, core,
    slot_idx); count <= slot size, zero-padded on device.
    """
    experts = [e for e in range(_E) if counts[e] > 0]

    from itertools import product

    best = None
    for sizes in slot_sizes_list:
        nslot = len(sizes)
        # options per expert: tuples (k_0..k_{nslot-1}) with sum(k_i*s_i) >= n
        def expert_opts(n):
            opts = [
                t for t in product(range(9), repeat=nslot)
                if sum(k * s for k, s in zip(t, sizes)) >= n
            ]
            return sorted(opts, key=lambda t: (sum(t), t))[:64]

        states = {tuple([0] * nslot): []}
        ok = True
        for e in experts:
            nxt = {}
            for opt in expert_opts(int(counts[e])):
                for st, hist in states.items():
                    ns = tuple(a + b for a, b in zip(st, opt))
                    if all(v <= 8 for v in ns) and ns not in nxt:
                        nxt[ns] = hist + [(e, opt)]
            if not nxt:
                ok = False
                break
            states = nxt
        if not ok:
            continue
        alloc = min(states.values(), key=lambda h: 0)  # any feasible
        cap = sum(sizes)
        pe_ns = cap * 218  # ~213ns/token PE + margin
        dma_ns = (nslot * 32e6 + cap * 8192) / 358e9 * 1e9
        score = max(pe_ns, dma_ns)
        if best is None or score < best[0]:
            best = (score, sizes, alloc)

    assert best is not None, f"no feasible slot structure for counts={counts}"
    _, sizes, alloc = best

    # materialize parts: per slot-kind, hand out slot indices core 0..7
    next_core = [0] * len(sizes)
    parts = []
    for e, opt in alloc:
        rem = int(counts[e])
        # fill largest slots first
        order = sorted(range(len(sizes)), key=lambda i: -sizes[i])
        for i in order:
            for _ in range(opt[i]):
                take = max(0, min(rem, sizes[i]))
                core = next_core[i]
                next_core[i] += 1
                parts.append((e, take, core, i))
                rem -= take
        assert rem <= 0
    return list(sizes), parts


_PROG_CACHE = {}


def _build_program(slot_sizes):
    """One SPMD Bass program for all 8 cores, parameterized by slot sizes."""
    import concourse.tile as tile
    from concourse import bacc, mybir

    key = tuple(slot_sizes)
    if key in _PROG_CACHE:
        return _PROG_CACHE[key]

    F32 = mybir.dt.float32
    F32R = mybir.dt.float32r
    CAP = sum(slot_sizes)
    nslot = len(slot_sizes)

    nc = bacc.Bacc("TRN2", target_bir_lowering=False, debug=False, num_devices=8)
    xt = nc.dram_tensor("xt", [128, _KD, CAP], F32, kind="ExternalInput").ap()
    w1d = [
        nc.dram_tensor(f"w1_{s}", [_HI, 128, _KD, 128], F32, kind="ExternalInput").ap()
        for s in range(nslot)
    ]
    w2d = [
        nc.dram_tensor(f"w2_{s}", [_KD, 128, _HI, 128], F32, kind="ExternalInput").ap()
        for s in range(nslot)
    ]
    b1d = [
        nc.dram_tensor(f"b1_{s}", [128, _HI], F32, kind="ExternalInput").ap()
        for s in range(nslot)
    ]
    b2d = [
        nc.dram_tensor(f"b2_{s}", [128, _KD], F32, kind="ExternalInput").ap()
        for s in range(nslot)
    ]
    outT = nc.dram_tensor("outT", [_KD, 128, CAP], F32, kind="ExternalOutput").ap()

    Relu = mybir.ActivationFunctionType.Relu
    Ident = mybir.ActivationFunctionType.Identity

    max_cks = max(len(_chunks(Ls)) for Ls in slot_sizes)
    offs = [0]
    for Ls in slot_sizes:
        offs.append(offs[-1] + Ls)
    with tile.TileContext(nc) as tc:
        with tc.tile_pool(name="xp", bufs=max(2, max_cks)) as xp, \
             tc.tile_pool(name="w1p", bufs=5) as w1p, \
             tc.tile_pool(name="w1f", bufs=2) as w1f, \
             tc.tile_pool(name="w2p", bufs=11) as w2p, \
             tc.tile_pool(name="h1p", bufs=1) as h1p, \
             tc.tile_pool(name="cp", bufs=2) as cp, \
             tc.tile_pool(name="op", bufs=4) as op, \
             tc.tile_pool(name="ps1", bufs=4, space="PSUM") as ps1, \
             tc.tile_pool(name="ps2", bufs=4, space="PSUM") as ps2:

            slot_pre = {}  # s -> (x tiles, w1-hi0 tile, b1 tile, b2 tile)

            def emit_slot_prefetch(s):
                """Queue slot s's x chunks, first w1 block, and biases on the
                sync ring. Order: x ck0, w1 hi0, biases, remaining x chunks —
                so the first matmul of the slot waits on as little as
                possible. For slot 0 (kernel startup, DMA-starved) the later
                x chunks are deferred and interleaved as k-halves between the
                first w1 blocks inside the L1 loop."""
                off = offs[s]
                cks = _chunks(slot_sizes[s])
                xc0 = xp.tile([128, _KD, cks[0][1]], F32R, tag="x")
                nc.sync.dma_start(
                    xc0[:], xt[:, :, off:off + cks[0][1]].bitcast(F32R)
                )
                w1_0 = w1f.tile([128, _KD, 128], F32R, tag="w1first")
                nc.sync.dma_start(w1_0[:], w1d[s][0].bitcast(F32R))
                b1_sb = cp.tile([128, _HI], F32, tag="b1")
                nc.sync.dma_start(b1_sb[:], b1d[s][:])
                b2_sb = cp.tile([128, _KD], F32, tag="b2")
                nc.sync.dma_start(b2_sb[:], b2d[s][:])
                xs = [xc0]
                pending = {}  # hi -> [(sbuf slice, dram slice)] for slot 0
                half = _KD // 2
                for ci, (t0, tl) in enumerate(cks[1:], start=1):
                    xc = xp.tile([128, _KD, tl], F32R, tag="x")
                    src = xt[:, :, off + t0:off + t0 + tl].bitcast(F32R)
                    if s == 0:
                        # both halves queued right after the w1-hi1 block:
                        # the PE needs w1-hi1 before any of chunk 1's x
                        pending.setdefault(1, []).append(
                            (xc[:, 0:half, :], src[:, 0:half, :])
                        )
                        pending.setdefault(1, []).append(
                            (xc[:, half:_KD, :], src[:, half:_KD, :])
                        )
                    else:
                        nc.sync.dma_start(xc[:], src)
                    xs.append(xc)
                slot_pre[s] = (xs, w1_0, b1_sb, b2_sb, pending)

            emit_slot_prefetch(0)
            for s, Ls in enumerate(slot_sizes):
                off = offs[s]
                cks = _chunks(Ls)
                x_sb, w1_first, b1_sb, b2_sb, pending_x = slot_pre.pop(s)

                h1_sb = h1p.tile([128, _HI, Ls], F32R, tag="h1")
                # (hi=0, chunk>0) runs at the END of layer 1: at slot start
                # only chunk 0's x has landed, so starting with (0,0) alone
                # avoids a PE stall waiting for the later x chunks
                l1_iter = [(0, 0)]
                l1_iter += [(hi, ci) for hi in range(1, _HI)
                            for ci in range(len(cks))]
                l1_iter += [(0, ci) for ci in range(1, len(cks))]
                w1_tiles = {0: w1_first}
                for hi, ci in l1_iter:
                    if hi not in w1_tiles:
                        w1_sb = w1p.tile([128, _KD, 128], F32R, tag="w1")
                        nc.sync.dma_start(w1_sb[:], w1d[s][hi].bitcast(F32R))
                        w1_tiles[hi] = w1_sb
                        # slot 0: deferred x-chunk halves ride between the
                        # first w1 blocks so the PE is never starved
                        for dst, src in pending_x.pop(hi, ()):
                            nc.sync.dma_start(dst, src)
                    w1_sb = w1_tiles[hi]
                    t0, tl = cks[ci]
                    ps = ps1.tile([128, tl], F32, tag="ps1")
                    for k in range(_KD):
                        nc.tensor.matmul(
                            ps[:], w1_sb[:, k, :], x_sb[ci][:, k, :],
                            start=(k == 0), stop=(k == _KD - 1),
                        )
                    nc.scalar.activation(
                        h1_sb[:, hi, t0:t0 + tl], ps[:], Relu,
                        bias=b1_sb[:, hi:hi + 1],
                    )

                for di in range(_KD):
                    # w2 streamed in quarter blocks (8 h-chunks each) so the
                    # hi-loop only waits on the quarter it needs
                    w2_sb = []
                    for q in range(4):
                        wq = w2p.tile([128, 8, 128], F32R, tag="w2")
                        nc.sync.dma_start(
                            wq[:], w2d[s][di, :, 8 * q:8 * q + 8, :].bitcast(F32R)
                        )
                        w2_sb.append(wq)
                    if di == _KD - 1 and s + 1 < nslot:
                        # all of this slot's weights are queued; prefetch the
                        # next slot's x/biases/first-w1 behind them
                        emit_slot_prefetch(s + 1)
                    for (t0, tl) in cks:
                        ps = ps2.tile([128, tl], F32, tag="ps2")
                        for hi in range(_HI):
                            nc.tensor.matmul(
                                ps[:], w2_sb[hi // 8][:, hi % 8, :],
                                h1_sb[:, hi, t0:t0 + tl],
                                start=(hi == 0), stop=(hi == _HI - 1),
                            )
                        ob = op.tile([128, tl], F32, tag="ob")
                        nc.scalar.activation(
                            ob[:], ps[:], Ident, bias=b2_sb[:, di:di + 1],
                        )
                        nc.sync.dma_start(
                            outT[di, :, off + t0:off + t0 + tl], ob[:]
                        )

    nc.compile()
    _PROG_CACHE[key] = nc
    return nc


def _run(x, gate_w, gate_b, w1, b1, w2, b2, trace=False, trace_cores=None):
    from concourse import bass_utils

    e_sel, scale = _route(x, gate_w, gate_b)
    counts = np.bincount(e_sel, minlength=_E)

    # candidate structures: all 2-slot (A>=B), plus guaranteed 3-slot fallbacks
    cands = []
    for A in range(256, 705, 16):
        for Bv in range(256, A + 1, 16):
            cands.append([A, Bv])
    cands.append([512, 512, 512])
    cands.append([704, 704, 704])
    slot_sizes, parts = _pack(counts, cands)
    CAP = sum(slot_sizes)

    # token ids per expert in sorted order
    order = np.argsort(e_sel, kind="stable")
    starts = np.zeros(_E + 1, np.int64)
    np.cumsum(counts, out=starts[1:])
    consumed = [0] * _E

    # slot offsets within a core's token axis
    offs = np.zeros(len(slot_sizes) + 1, np.int64)
    np.cumsum(slot_sizes, out=offs[1:])

    xs = x.reshape(_N, _D) * scale[:, None]  # f32, matches reference scaling

    # prearranged weights, one contiguous block per (expert, chunk):
    # W1L[e, hi, p, k, f] = w1[e, hi*128+f, k*128+p]
    W1L = np.ascontiguousarray(
        w1.reshape(_E, _HI, 128, _KD, 128).transpose(0, 1, 4, 3, 2)
    )
    # W2L[e, di, p, hi, f] = w2[e, di*128+f, hi*128+p]
    W2L = np.ascontiguousarray(
        w2.reshape(_E, _KD, 128, _HI, 128).transpose(0, 1, 4, 3, 2)
    )
    B1L = np.ascontiguousarray(b1.reshape(_E, _HI, 128).transpose(0, 2, 1))
    B2L = np.ascontiguousarray(b2.reshape(_E, _KD, 128).transpose(0, 2, 1))

    slot_expert = [[0] * len(slot_sizes) for _ in range(8)]
    tok_of = np.full((8, CAP), -1, np.int64)
    for (e, cnt, core, si) in parts:
        lo = starts[e] + consumed[e]
        consumed[e] += cnt
        toks = order[lo:lo + cnt]
        tok_of[core, offs[si]:offs[si] + cnt] = toks
        slot_expert[core][si] = e

    in_maps = []
    for core in range(8):
        cols = tok_of[core]
        xsel = np.zeros((CAP, _D), np.float32)
        valid = cols >= 0
        xsel[valid] = xs[cols[valid]]
        XL = np.ascontiguousarray(
            xsel.reshape(CAP, _KD, 128).transpose(2, 1, 0)
        )
        m = {"xt": XL}
        for si in range(len(slot_sizes)):
            e = slot_expert[core][si]
            m[f"w1_{si}"] = W1L[e]
            m[f"w2_{si}"] = W2L[e]
            m[f"b1_{si}"] = B1L[e]
            m[f"b2_{si}"] = B2L[e]
        in_maps.append(m)

    nc = _build_program(slot_sizes)
    kw = {}
    if trace:
        kw["trace"] = True
        if trace_cores is not None:
            kw["trace_cores"] = trace_cores
    try:
        res = bass_utils.run_bass_kernel_spmd(
            nc, in_maps, core_ids=list(range(8)), **kw
        )
    except Exception:
        # one retry for transient device faults
        import time as _time
        _time.sleep(2.0)
        res = bass_utils.run_bass_kernel_spmd(
            nc, in_maps, core_ids=list(range(8)), **kw
        )

    out = np.zeros((_N, _D), np.float32)
    for core in range(8):
        cols = tok_of[core]
        valid = cols >= 0
        oc = res.results[core]["outT"]  # [KD, 128, CAP]
        ovals = oc.transpose(2, 0, 1).reshape(CAP, _D)
        out[cols[valid]] = ovals[valid]
    return out.reshape(_B, _S, _D), res


def kernel(x, gate_w, gate_b, w1, b1, w2, b2):
    x = np.ascontiguousarray(np.asarray(x, dtype=np.float32))
    gate_w = np.asarray(gate_w, dtype=np.float32)
    gate_b = np.asarray(gate_b, dtype=np.float32)
    w1 = np.ascontiguousarray(np.asarray(w1, dtype=np.float32))
    b1 = np.asarray(b1, dtype=np.float32)
    w2 = np.ascontiguousarray(np.asarray(w2, dtype=np.float32))
    b2 = np.asarray(b2, dtype=np.float32)
    out, _ = _run(x, gate_w, gate_b, w1, b1, w2, b2)
    return out
